# revision 7
# baseline (speedup 1.0000x reference)
# CWVAE (3-level RSSM scan) Trainium2 kernel — single NeuronCore.
#
# Strategy:
#  * All matmuls bf16 x bf16 -> fp32 PSUM. Batch (B=32) rides the PE stationary
#    operand; weights stream. 4x column tiling (128x32 tiles) fills the array.
#  * Activations live in "quartered" layout: SBUF [128, 256] where partition
#    32*q + b holds hidden dims [256q, 256q+256) of batch sample b.
#  * PE transposes (identity matmul) produce the [K,32] lhsT blocks needed by
#    the next matmul in the recurrence.
#  * qmean folded into next step's h1 via W_fuse = W_ps @ qm_w (host-computed),
#    so the carried state is (qh, det) and qmean is recovered in the postpass.
#  * obs/context contributions to h1/qh are precomputed outside the scan
#    (obs_part / c_part) as M-batched matmuls; heads (pmean/pstd/qmean/qstd)
#    are computed in an M-batched postpass from stored transposed det/qh.
import numpy as np
import ml_dtypes
from contextlib import ExitStack

import concourse.bass as bass
import concourse.tile as tile
from concourse import mybir
from concourse.masks import make_identity

F32 = mybir.dt.float32
BF16 = mybir.dt.bfloat16
NBF = ml_dtypes.bfloat16

B = 32
D = 1024          # deter
S = 256           # stoch
E = 1024          # emb
NQ = 4            # quarters
QD = D // NQ      # 256
KB = D // 128     # 8 K-blocks of the 1024-dim contractions
MIN_STD = 1e-4
SP_BIAS = 0.54


def bfc(x):
    return np.ascontiguousarray(x.astype(NBF))


def pack_quartered(WT):
    """WT: [K, N] (K contraction, N output) -> [K//128, NQ, 128, N//NQ]
    tile (k, j) = WT[128k:128k+128, (N//NQ)*j : (N//NQ)*(j+1)]"""
    K, N = WT.shape
    nj = N // NQ
    out = np.empty((K // 128, NQ, 128, nj), WT.dtype)
    for k in range(K // 128):
        for j in range(NQ):
            out[k, j] = WT[128 * k:128 * (k + 1), nj * j:nj * (j + 1)]
    return np.ascontiguousarray(out)


def prep_inputs(inputs, T0=64):
    """Host-side: cast/permute weights into SBUF tile layouts. Returns dict."""
    Ts = [T0, T0 // 4, T0 // 16]
    d = {}
    for l in range(3):
        ph1 = inputs["ph1_w"][l].astype(np.float32)       # [E, S+D]
        W_ps = ph1[:, :S]                                  # [E, S]
        W_ctx = ph1[:, S:]                                 # [E, D]
        qm = inputs["qmean_w"][l].astype(np.float32)       # [S, E]
        W_fuse = (W_ps.astype(np.float64) @ qm.astype(np.float64)).astype(np.float32)  # [E, E]
        wihT = inputs["gru_wih"][l].astype(np.float32).T   # [E, 3D]
        whhT = inputs["gru_whh"][l].astype(np.float32).T   # [D, 3D]
        wqdT = inputs["qh1_w"][l][:, :D].astype(np.float32).T    # [D, E]
        wqoT = inputs["qh1_w"][l][:, D:].astype(np.float32).T    # [E(obs), E]
        wctxT = W_ctx.T                                    # [D, E]
        wfuseT = W_fuse.T                                  # [E(qh), E(h1)]

        def rz(WT):  # [K, 3D] -> rz tiles [K//128, 4, 128, 512]
            K = WT.shape[0]
            out = np.empty((K // 128, NQ, 128, 2 * QD), np.float32)
            for k in range(K // 128):
                for j in range(NQ):
                    out[k, j, :, :QD] = WT[128 * k:128 * (k + 1), QD * j:QD * (j + 1)]
                    out[k, j, :, QD:] = WT[128 * k:128 * (k + 1), D + QD * j:D + QD * (j + 1)]
            return out

        def ngate(WT):
            K = WT.shape[0]
            out = np.empty((K // 128, NQ, 128, QD), np.float32)
            for k in range(K // 128):
                for j in range(NQ):
                    out[k, j] = WT[128 * k:128 * (k + 1), 2 * D + QD * j:2 * D + QD * (j + 1)]
            return out

        d[f"wihrz{l}"] = bfc(rz(wihT))
        d[f"wihn{l}"] = bfc(ngate(wihT))
        d[f"whhrz{l}"] = bfc(rz(whhT))
        d[f"whhn{l}"] = bfc(ngate(whhT))
        d[f"wqd{l}"] = bfc(pack_quartered(wqdT))
        d[f"wfuse{l}"] = bfc(pack_quartered(wfuseT))
        d[f"wqo{l}"] = bfc(np.ascontiguousarray(wqoT.reshape(KB, 128, E)))
        if l < 2:
            d[f"wctx{l}"] = bfc(pack_quartered(wctxT))
        obs = inputs[f"obs_l{l}"].astype(np.float32)       # [B, T, E]
        d[f"obs{l}"] = bfc(obs)
    # postpass heads, packed as one [4, 8, 128, 256] (head, k, p, n): pm, ps, qm, qs
    post = np.stack([
        np.ascontiguousarray(inputs["pmean_w"][0].astype(np.float32).T.reshape(KB, 128, S)),
        np.ascontiguousarray(inputs["pstd_w"][0].astype(np.float32).T.reshape(KB, 128, S)),
        np.ascontiguousarray(inputs["qmean_w"][0].astype(np.float32).T.reshape(KB, 128, S)),
        np.ascontiguousarray(inputs["qstd_w"][0].astype(np.float32).T.reshape(KB, 128, S)),
    ])
    d["wpost"] = bfc(post)
    return d


INPUT_SPECS = None  # filled by build()


def lhs_blk(tT, k):
    """transposed-activation SBUF tile [128, 2, 128] -> lhsT block k [128, 32]"""
    s, q = k % 2, k // 2
    return tT[:, s, 32 * q:32 * (q + 1)]


def build(ctx: ExitStack, tc: tile.TileContext, outs, ins, T0=64):
    nc = tc.nc
    Ts = [T0, T0 // 4, T0 // 16]
    out_f = outs["out_f"]

    const = ctx.enter_context(tc.tile_pool(name="const", bufs=1))
    wpool = ctx.enter_context(tc.tile_pool(name="wpool", bufs=1))
    cpool = ctx.enter_context(tc.tile_pool(name="cpool", bufs=2))
    spool = ctx.enter_context(tc.tile_pool(name="spool", bufs=3))
    work = ctx.enter_context(tc.tile_pool(name="work", bufs=2))
    pps = ctx.enter_context(tc.tile_pool(name="pps", bufs=1, space="PSUM"))
    dpool = ctx.enter_context(tc.tile_pool(name="dpool", bufs=1, space="DRAM"))

    ident = const.tile([128, 128], BF16)
    make_identity(nc, ident)
    sp_bias = const.tile([128, 1], F32)
    nc.vector.memset(sp_bias, SP_BIAS)

    # DRAM intermediates (via tracked DRAM pool tiles)
    obs_part_d = [dpool.tile([Ts[l], 128, QD], F32, tag=f"obs_part{l}", name=f"obs_part{l}")
                  for l in range(3)]
    c_part_d = {l: dpool.tile([Ts[l], 128, QD], F32, tag=f"c_part{l}", name=f"c_part{l}")
                for l in range(2)}
    detT_d = [dpool.tile([Ts[l], 128, 2, 128], BF16, tag=f"detT{l}", name=f"detT{l}")
              for l in range(3)]
    qhT_d = None  # level-0 qh/det transposes stored postpass-friendly instead
    detTp_d = dpool.tile([Ts[0] // 4, 128, 2, NQ, 4, 32], BF16, tag="detTp", name="detTp")
    preact_d = dpool.tile([Ts[0] // 4, 128, 2 * S], F32, tag="preact", name="preact")
    qhTp_d = dpool.tile([Ts[0] // 4, 128, 2, NQ, 4, 32], BF16, tag="qhTp", name="qhTp")

    # ---------- obs_part emission (levels 2,1 upfront; level 0 interleaved) ----
    def emit_obs_mtile(l, m, wqo):
        obs_flat = ins[f"obs{l}"].flatten_outer_dims()      # [B*T, E]
        R = B * Ts[l]
        Tl = Ts[l]
        mrows = min(128, R - 128 * m)
        obs_n = work.tile([128, KB, 128], BF16, tag="obs_n", name="obs_n")
        nc.gpsimd.dma_start(out=obs_n[:mrows].rearrange("p k f -> p (k f)"),
                            in_=obs_flat[128 * m:128 * m + mrows, :])
        obsT_m = work.tile([128, KB, 128], BF16, tag="obsT", name="obsT_m")
        for k in range(KB):
            pt = pps.tile([128, 128], BF16, tag="p_t", bufs=2, name="pt_obs")
            nc.tensor.transpose(pt[:, :mrows], obs_n[:mrows, k, :], ident[:mrows, :mrows])
            nc.vector.tensor_copy(obsT_m[:, k, :mrows], pt[:, :mrows])
        p0 = pps.tile([128, 512], F32, tag="p_rz", name="p0", bufs=2)
        p1 = pps.tile([128, 512], F32, tag="p_h1", name="p1")
        for k in range(KB):
            nc.tensor.matmul(p0[:mrows], obsT_m[:, k, :mrows], wqo[:, k, 0:512],
                             start=(k == 0), stop=(k == KB - 1))
            nc.tensor.matmul(p1[:mrows], obsT_m[:, k, :mrows], wqo[:, k, 512:1024],
                             start=(k == 0), stop=(k == KB - 1))
        osb = work.tile([128, E], F32, tag="opart_sb", name="osb", bufs=1)
        nc.vector.tensor_copy(osb[:mrows, 0:512], p0[:mrows])
        nc.scalar.copy(osb[:mrows, 512:1024], p1[:mrows])
        nb = mrows // Tl
        b_base = (128 * m) // Tl
        dest = obs_part_d[l].rearrange("t (q b) f -> b t q f", q=NQ)
        osb_v = osb.rearrange("p (q f) -> p q f", q=NQ)
        for bb in range(nb):
            eng = nc.sync
            eng.dma_start(out=dest[b_base + bb],
                          in_=osb_v[Tl * bb:Tl * (bb + 1)])

    def load_wqo(l):
        wqo = wpool.tile([128, KB, E], BF16, tag="big1", name=f"wqo{l}")
        for k in range(KB):
            nc.gpsimd.dma_start(out=wqo[:, k, :], in_=ins[f"wqo{l}"][k])
        return wqo

    for l in (2, 1):
        wqo = load_wqo(l)
        for m in range((B * Ts[l] + 127) // 128):
            emit_obs_mtile(l, m, wqo)

    wpost_box = [None]

    post_state = {}

    def emit_post_head(m, h):
        # one head of postpass M-tile m (level-0 steps 4m..4m+3), emitted as PE
        # filler in the gates window of a scan step. pmean/qmean finish here;
        # pstd/qstd staged for the epilogue (avoids ACT table thrash mid-scan).
        wpost = wpost_box[0]
        if h == 0:
            dT4 = work.tile([128, 2, NQ, 4, 32], BF16, tag="dT4", bufs=2, name="dT4")
            qT4 = work.tile([128, 2, NQ, 4, 32], BF16, tag="qT4", bufs=2, name="qT4")
            nc.sync.dma_start(out=dT4, in_=detTp_d[m])
            nc.sync.dma_start(out=qT4, in_=qhTp_d[m])
            pa = work.tile([128, 2 * S], F32, tag="pa", bufs=2, name="pa")
            post_state[m] = (dT4, qT4, pa)
        dT4, qT4, pa = post_state[m]
        src_t = dT4 if h in (0, 1) else qT4
        ph = pps.tile([128, S], F32, tag="p_h1", name=f"post{h}")
        for k in range(KB):
            s, q = k % 2, k // 2
            lhsT = src_t[:, s, q].rearrange("p a b -> p (a b)")
            nc.tensor.matmul(ph, lhsT, wpost[:, h, k, :],
                             start=(k == 0), stop=(k == KB - 1))
        if h in (0, 2):
            hsb = work.tile([128, S], F32, tag="hsb", name="hsb")
            nc.vector.tensor_copy(hsb, ph)
            col = D if h == 0 else D + 2 * S
            dest = out_f[:, 4 * m:4 * (m + 1), col:col + S]
            nc.sync.dma_start(out=dest.rearrange("b t e -> t b e"), in_=hsb)
        else:
            nc.vector.tensor_copy(pa[:, (h // 2) * S:(h // 2) * S + S], ph)
            if h == 3:
                nc.sync.dma_start(out=preact_d[m], in_=pa)
                del post_state[m]

    # ---------- level loop ----------
    for l in (2, 1, 0):
        T = Ts[l]
        top = (l == 2)
        # c_part phase for this level (from parent's stored detT)
        if not top:
            wctx = wpool.tile([128, KB, NQ, QD], BF16, tag="big1", name=f"wctx{l}")
            for k in range(KB):
                nc.gpsimd.dma_start(out=wctx[:, k], in_=ins[f"wctx{l}"][k].rearrange("j p n -> p j n"))
            for p in range(Ts[l + 1]):
                plhsT = work.tile([128, 2, 128], BF16, tag="pstep_lhsT")
                nc.sync.dma_start(out=plhsT, in_=detT_d[l + 1][p])
                pq = pps.tile([128, QD], F32, tag="p_qh", name="pq_cp")
                for k in range(KB):
                    for j in range(NQ):
                        nc.tensor.matmul(pq[32 * j:32 * (j + 1), :], lhs_blk(plhsT, k),
                                         wctx[:, k, j, :], start=(k == 0), stop=(k == KB - 1),
                                         tile_position=(0, 32 * j), skip_group_check=True)
                csb = work.tile([128, QD], F32, tag="cpart_sb")
                nc.vector.tensor_copy(csb, pq)
                for i in range(4):
                    nc.sync.dma_start(out=c_part_d[l][4 * p + i], in_=csb)
        wqo0 = None
        if l == 1:
            wqo0 = load_wqo(0)
            n_obs0_mtiles = (B * Ts[0] + 127) // 128
        if l == 0:
            wpost = wpool.tile([128, 4, KB, S], BF16, tag="big1", name="wpost")
            for h in range(4):
                for k in range(KB):
                    nc.gpsimd.dma_start(out=wpost[:, h, k, :], in_=ins["wpost"][h, k])
            wpost_box[0] = wpost

        # level weights
        wihrz = wpool.tile([128, KB, NQ, 2 * QD], BF16, tag="wihrz", name=f"wihrz{l}")
        whhrz = wpool.tile([128, KB, NQ, 2 * QD], BF16, tag="whhrz", name=f"whhrz{l}")
        wihn = wpool.tile([128, KB, NQ, QD], BF16, tag="wihn", name=f"wihn{l}")
        whhn = wpool.tile([128, KB, NQ, QD], BF16, tag="whhn", name=f"whhn{l}")
        wqd = wpool.tile([128, KB, NQ, QD], BF16, tag="wqd", name=f"wqd{l}")
        wfuse = wpool.tile([128, KB, NQ, QD], BF16, tag="wfuse", name=f"wfuse{l}")
        for k in range(KB):
            nc.gpsimd.dma_start(out=wihrz[:, k], in_=ins[f"wihrz{l}"][k].rearrange("j p n -> p j n"))
            nc.gpsimd.dma_start(out=whhrz[:, k], in_=ins[f"whhrz{l}"][k].rearrange("j p n -> p j n"))
            nc.gpsimd.dma_start(out=wihn[:, k], in_=ins[f"wihn{l}"][k].rearrange("j p n -> p j n"))
            nc.gpsimd.dma_start(out=whhn[:, k], in_=ins[f"whhn{l}"][k].rearrange("j p n -> p j n"))
            nc.gpsimd.dma_start(out=wqd[:, k], in_=ins[f"wqd{l}"][k].rearrange("j p n -> p j n"))
            nc.gpsimd.dma_start(out=wfuse[:, k], in_=ins[f"wfuse{l}"][k].rearrange("j p n -> p j n"))

        detf_c = detT_c = qhT_c = None
        qhbf_pend = None
        for t in range(T):
            first = (t == 0)
            has_ctx = not top
            if not top and l == 1 and wqo0 is not None and t < n_obs0_mtiles:
                emit_obs_mtile(0, t, wqo0)

            # stream tiles
            cpt = None
            if has_ctx:
                cpt = spool.tile([128, QD], F32, tag="cpt")
                nc.sync.dma_start(out=cpt, in_=c_part_d[l][t])
            opt = spool.tile([128, QD], F32, tag="opt")
            nc.sync.dma_start(out=opt, in_=obs_part_d[l][t])

            # --- gh first: independent of this step's chain, fills PE queue
            do_gh = not first
            prz = pgin = pghn = None
            if do_gh or not (top and first):
                prz = pps.tile([128, 2 * QD], F32, tag="p_rz", name="prz", bufs=2)
                pgin = pps.tile([128, QD], F32, tag="p_gin", name="pgin")
                pghn = pps.tile([128, QD], F32, tag="p_ghn", name="pghn")
            if do_gh:
                for k in range(KB):
                    for j in range(NQ):
                        nc.tensor.matmul(prz[32 * j:32 * (j + 1), :], lhs_blk(detT_c, k),
                                         whhrz[:, k, j, :], start=(k == 0),
                                         stop=False,
                                         tile_position=(0, 32 * j), skip_group_check=True)
                    for j in range(NQ):
                        nc.tensor.matmul(pghn[32 * j:32 * (j + 1), :], lhs_blk(detT_c, k),
                                         whhn[:, k, j, :], start=(k == 0), stop=(k == KB - 1),
                                         tile_position=(0, 32 * j), skip_group_check=True)
            # --- deferred T(qh) from previous step (behind gh in the PE queue)
            if qhbf_pend is not None:
                qhT_c = cpool.tile([128, 2, 128], BF16, tag="qhT", name="qhT_c")
                for s in range(2):
                    pt = pps.tile([128, 128], BF16, tag="p_t", bufs=2, name="pt_q")
                    nc.tensor.transpose(pt, qhbf_pend[:, 128 * s:128 * (s + 1)], ident)
                    # split the two PSUM->SBUF copies across DVE/ACT so they
                    # overlap instead of serializing on the vector engine
                    if s == 0:
                        nc.vector.tensor_copy(qhT_c[:, s, :], pt)
                    else:
                        nc.scalar.copy(qhT_c[:, s, :], pt)
                if l == 0 and t > 0:
                    nc.sync.dma_start(
                        out=qhTp_d[(t - 1) // 4][:, :, :, (t - 1) % 4, :],
                        in_=qhT_c.rearrange("p s (q b) -> p s q b", q=NQ))
                qhbf_pend = None
            # --- MM-A: h1 pre-activation from qh carry (fused weights)
            ph1 = None
            if not first:
                ph1 = pps.tile([128, QD], F32, tag="p_h1", name="ph1")
                for k in range(KB):
                    for j in range(NQ):
                        nc.tensor.matmul(ph1[32 * j:32 * (j + 1), :], lhs_blk(qhT_c, k),
                                         wfuse[:, k, j, :], start=(k == 0), stop=(k == KB - 1),
                                         tile_position=(0, 32 * j), skip_group_check=True)
            # --- h1 (bf16, relu)
            h1bf = None
            if ph1 is not None and cpt is not None:
                h1bf = work.tile([128, QD], BF16, tag="h1bf")
                nc.vector.tensor_add(h1bf, ph1, cpt)
                nc.vector.tensor_scalar_max(h1bf, h1bf, 0.0)
            elif ph1 is not None:
                h1bf = work.tile([128, QD], BF16, tag="h1bf")
                nc.vector.tensor_scalar_max(h1bf, ph1, 0.0)
            elif cpt is not None:
                h1bf = work.tile([128, QD], BF16, tag="h1bf")
                nc.vector.tensor_scalar_max(h1bf, cpt, 0.0)
            # --- T(h1)
            h1T = None
            if h1bf is not None:
                h1T = work.tile([128, 2, 128], BF16, tag="h1T")
                for s in range(2):
                    pt = pps.tile([128, 128], BF16, tag="p_t", bufs=2, name="pt_h1")
                    nc.tensor.transpose(pt, h1bf[:, 128 * s:128 * (s + 1)], ident)
                    if s == 0:
                        nc.vector.tensor_copy(h1T[:, s, :], pt)
                    else:
                        nc.scalar.copy(h1T[:, s, :], pt)

            # --- GRU (gi; gh already emitted above)
            do_gi = h1T is not None
            detf_new = cpool.tile([128, QD], F32, tag="detf")
            detbf = None
            if do_gh or do_gi:
                if do_gi:
                    for k in range(KB):
                        for j in range(NQ):
                            nc.tensor.matmul(prz[32 * j:32 * (j + 1), :], lhs_blk(h1T, k),
                                             wihrz[:, k, j, :], start=(k == 0) and not do_gh,
                                             stop=(k == KB - 1),
                                             tile_position=(0, 32 * j), skip_group_check=True)
                        for j in range(NQ):
                            nc.tensor.matmul(pgin[32 * j:32 * (j + 1), :], lhs_blk(h1T, k),
                                             wihn[:, k, j, :], start=(k == 0), stop=(k == KB - 1),
                                             tile_position=(0, 32 * j), skip_group_check=True)
                if l == 0 and t >= 4:
                    emit_post_head((t - 4) // 4, (t - 4) % 4)
                # HAM keep-warm filler: the gates' ACT/DVE chain leaves the PE
                # idle ~3us each step, re-throttling the clock gate to K=4/8
                # (all matmuls then run at 1.2GHz). Burn ~1.7us of dummy
                # streams into the recycled p_qh bank (never read; the real
                # qh matmul later start=True-clears it) to keep K=8/8.
                if detT_c is not None:
                    pdum = pps.tile([128, QD], F32, tag="p_qh", name="pdum")
                    for rep in range(3):
                        for k in range(KB):
                            for j in range(NQ):
                                nc.tensor.matmul(
                                    pdum[32 * j:32 * (j + 1), :],
                                    lhs_blk(detT_c, k),
                                    whhrz[:, k, j, 0:QD],
                                    start=(k == 0), stop=(k == KB - 1),
                                    tile_position=(0, 32 * j),
                                    skip_group_check=True)
                # gates
                r_s = work.tile([128, QD], F32, tag="r_s")
                nc.scalar.activation(r_s, prz[:, 0:QD], mybir.ActivationFunctionType.Sigmoid)
                if do_gh and do_gi:
                    t1 = work.tile([128, QD], F32, tag="t1")
                    nc.vector.tensor_mul(t1, r_s, pghn)
                    nc.vector.tensor_add(t1, t1, pgin)
                    n_in = t1
                elif do_gi:
                    n_in = pgin
                else:
                    t1 = work.tile([128, QD], F32, tag="t1")
                    nc.vector.tensor_mul(t1, r_s, pghn)
                    n_in = t1
                z_s = work.tile([128, QD], F32, tag="z_s")
                nc.scalar.activation(z_s, prz[:, QD:2 * QD], mybir.ActivationFunctionType.Sigmoid)
                n_s = work.tile([128, QD], F32, tag="n_s")
                nc.scalar.activation(n_s, n_in, mybir.ActivationFunctionType.Tanh)
                # det' = n*(1-z) + z*det ; omz/zdet overlap the tanh
                omz = work.tile([128, QD], F32, tag="omz")
                nc.vector.tensor_scalar(omz, z_s, -1.0, 1.0,
                                        mybir.AluOpType.mult, mybir.AluOpType.add)
                d1 = work.tile([128, QD], F32, tag="d1")
                if not first:
                    nc.vector.tensor_mul(d1, z_s, detf_c)
                    nc.vector.tensor_mul(omz, omz, n_s)
                    nc.vector.tensor_add(detf_new, omz, d1)
                else:
                    nc.vector.tensor_mul(detf_new, omz, n_s)
                detbf = work.tile([128, QD], BF16, tag="detbf")
                nc.vector.tensor_copy(detbf, detf_new)
            else:
                nc.vector.memset(detf_new, 0.0)

            # --- T(det)
            detT_new = cpool.tile([128, 2, 128], BF16, tag="detT")
            if detbf is not None:
                for s in range(2):
                    pt = pps.tile([128, 128], BF16, tag="p_t", bufs=2, name="pt_d")
                    nc.tensor.transpose(pt, detbf[:, 128 * s:128 * (s + 1)], ident)
                    if s == 0:
                        nc.vector.tensor_copy(detT_new[:, s, :], pt)
                    else:
                        nc.scalar.copy(detT_new[:, s, :], pt)
            else:
                nc.vector.memset(detT_new, 0.0)
            if l == 0:
                nc.sync.dma_start(
                    out=detTp_d[t // 4][:, :, :, t % 4, :],
                    in_=detT_new.rearrange("p s (q b) -> p s q b", q=NQ))
                dest = out_f[:, t, 0:D].rearrange("b (q f) -> q b f", q=NQ)
                nc.sync.dma_start(out=dest, in_=detf_new)
            else:
                nc.sync.dma_start(out=detT_d[l][t], in_=detT_new)

            # --- qh
            pqh = None
            if detbf is not None:
                pqh = pps.tile([128, QD], F32, tag="p_qh", name="pqh")
                for k in range(KB):
                    for j in range(NQ):
                        nc.tensor.matmul(pqh[32 * j:32 * (j + 1), :], lhs_blk(detT_new, k),
                                         wqd[:, k, j, :], start=(k == 0), stop=(k == KB - 1),
                                         tile_position=(0, 32 * j), skip_group_check=True)
            # keep-warm filler across the step boundary (qhbf DVE ops + next
            # step's stream DMAs leave the PE idle ~1-2us)
            if detbf is not None:
                pdum2 = pps.tile([128, QD], F32, tag="p_gin", name="pdum2")
                for k in range(KB):
                    for j in range(NQ):
                        nc.tensor.matmul(
                            pdum2[32 * j:32 * (j + 1), :],
                            lhs_blk(detT_new, k),
                            whhn[:, k, j, :],
                            start=(k == 0), stop=(k == KB - 1),
                            tile_position=(0, 32 * j),
                            skip_group_check=True)
            qhbf = work.tile([128, QD], BF16, tag="qhbf")
            if pqh is not None:
                nc.vector.tensor_add(qhbf, pqh, opt)
                nc.vector.tensor_scalar_max(qhbf, qhbf, 0.0)
            else:
                nc.vector.tensor_scalar_max(qhbf, opt, 0.0)
            qhbf_pend = qhbf
            detf_c, detT_c = detf_new, detT_new

    # final deferred T(qh) of the last level-0 step (feeds qhTp store)
    if qhbf_pend is not None:
        qhT_c = cpool.tile([128, 2, 128], BF16, tag="qhT", name="qhT_last")
        for s in range(2):
            pt = pps.tile([128, 128], BF16, tag="p_t", bufs=2, name="pt_ql")
            nc.tensor.transpose(pt, qhbf_pend[:, 128 * s:128 * (s + 1)], ident)
            nc.vector.tensor_copy(qhT_c[:, s, :], pt)
        nc.sync.dma_start(
            out=qhTp_d[(Ts[0] - 1) // 4][:, :, :, (Ts[0] - 1) % 4, :],
            in_=qhT_c.rearrange("p s (q b) -> p s q b", q=NQ))
    for h in range(4):
        emit_post_head(Ts[0] // 4 - 1, h)

    # ---------- level-0 heads epilogue: softplus of staged pre-activations ----
    for m in range(Ts[0] // 4):
        pa = work.tile([128, 2 * S], F32, tag="pa_e")
        nc.sync.dma_start(out=pa, in_=preact_d[m])
        pe_ = work.tile([128, 2 * S], F32, tag="pe_e")
        nc.scalar.activation(pe_, pa, mybir.ActivationFunctionType.Exp, bias=sp_bias)
        nc.scalar.activation(pe_, pe_, mybir.ActivationFunctionType.Ln, bias=1.0)
        nc.vector.tensor_scalar_add(pe_, pe_, MIN_STD)
        d0 = out_f[:, 4 * m:4 * (m + 1), D + S:D + 2 * S]
        nc.sync.dma_start(out=d0.rearrange("b t e -> t b e"), in_=pe_[:, 0:S])
        d1_ = out_f[:, 4 * m:4 * (m + 1), D + 3 * S:D + 4 * S]
        nc.sync.dma_start(out=d1_.rearrange("b t e -> t b e"), in_=pe_[:, S:2 * S])


# ------------------------- runner -------------------------
_CACHE = {}


def _get_program(T0):
    if T0 in _CACHE:
        return _CACHE[T0]
    from concourse import bacc
    nc = bacc.Bacc("TRN2", target_bir_lowering=False, debug=False, num_devices=1)
    in_specs = _input_specs(T0)
    ins = {k: nc.dram_tensor(k, list(shape), dt, kind="ExternalInput").ap()
           for k, (shape, dt) in in_specs.items()}
    outs = {"out_f": nc.dram_tensor("out_f", [B, T0, D + 4 * S], F32,
                                    kind="ExternalOutput").ap()}
    with tile.TileContext(nc) as tc:
        with ExitStack() as ctx:
            build(ctx, tc, outs, ins, T0=T0)
    nc.compile()
    _CACHE[T0] = nc
    return nc


def _input_specs(T0):
    Ts = [T0, T0 // 4, T0 // 16]
    sp = {}
    for l in range(3):
        sp[f"wihrz{l}"] = ([KB, NQ, 128, 2 * QD], BF16)
        sp[f"wihn{l}"] = ([KB, NQ, 128, QD], BF16)
        sp[f"whhrz{l}"] = ([KB, NQ, 128, 2 * QD], BF16)
        sp[f"whhn{l}"] = ([KB, NQ, 128, QD], BF16)
        sp[f"wqd{l}"] = ([KB, NQ, 128, QD], BF16)
        sp[f"wfuse{l}"] = ([KB, NQ, 128, QD], BF16)
        sp[f"wqo{l}"] = ([KB, 128, E], BF16)
        if l < 2:
            sp[f"wctx{l}"] = ([KB, NQ, 128, QD], BF16)
        sp[f"obs{l}"] = ([B, Ts[l], E], BF16)
    sp["wpost"] = ([4, KB, 128, S], BF16)
    return sp


def run(inputs, trace=False):
    from concourse.bass_utils import run_bass_kernel_spmd
    inputs = {k: np.asarray(v) for k, v in inputs.items()}
    T0 = int(inputs["obs_l0"].shape[1])
    prepped = prep_inputs(inputs, T0)
    nc = _get_program(T0)
    res = run_bass_kernel_spmd(nc, [prepped], core_ids=[0], trace=trace)
    out = res.results[0]["out_f"].astype(np.float32)
    return out, res


def kernel(**inputs):
    out, _ = run(inputs, trace=False)
    return out



# revision 10
# speedup vs baseline: 1.0211x; 1.0211x over previous
# CWVAE (3-level RSSM scan) Trainium2 kernel — single NeuronCore.
#
# Strategy:
#  * All matmuls bf16 x bf16 -> fp32 PSUM. Batch (B=32) rides the PE stationary
#    operand; weights stream. 4x column tiling (128x32 tiles) fills the array.
#  * Activations live in "quartered" layout: SBUF [128, 256] where partition
#    32*q + b holds hidden dims [256q, 256q+256) of batch sample b.
#  * PE transposes (identity matmul) produce the [K,32] lhsT blocks needed by
#    the next matmul in the recurrence.
#  * qmean folded into next step's h1 via W_fuse = W_ps @ qm_w (host-computed),
#    so the carried state is (qh, det) and qmean is recovered in the postpass.
#  * obs/context contributions to h1/qh are precomputed outside the scan
#    (obs_part / c_part) as M-batched matmuls; heads (pmean/pstd/qmean/qstd)
#    are computed in an M-batched postpass from stored transposed det/qh.
import numpy as np
import ml_dtypes
from contextlib import ExitStack

import concourse.bass as bass
import concourse.tile as tile
from concourse import mybir
from concourse.masks import make_identity

F32 = mybir.dt.float32
BF16 = mybir.dt.bfloat16
NBF = ml_dtypes.bfloat16

B = 32
D = 1024          # deter
S = 256           # stoch
E = 1024          # emb
NQ = 4            # quarters
QD = D // NQ      # 256
KB = D // 128     # 8 K-blocks of the 1024-dim contractions
MIN_STD = 1e-4
SP_BIAS = 0.54


def bfc(x):
    return np.ascontiguousarray(x.astype(NBF))


def pack_quartered(WT):
    """WT: [K, N] (K contraction, N output) -> [K//128, NQ, 128, N//NQ]
    tile (k, j) = WT[128k:128k+128, (N//NQ)*j : (N//NQ)*(j+1)]"""
    K, N = WT.shape
    nj = N // NQ
    out = np.empty((K // 128, NQ, 128, nj), WT.dtype)
    for k in range(K // 128):
        for j in range(NQ):
            out[k, j] = WT[128 * k:128 * (k + 1), nj * j:nj * (j + 1)]
    return np.ascontiguousarray(out)


def prep_inputs(inputs, T0=64):
    """Host-side: cast/permute weights into SBUF tile layouts. Returns dict."""
    Ts = [T0, T0 // 4, T0 // 16]
    d = {}
    for l in range(3):
        ph1 = inputs["ph1_w"][l].astype(np.float32)       # [E, S+D]
        W_ps = ph1[:, :S]                                  # [E, S]
        W_ctx = ph1[:, S:]                                 # [E, D]
        qm = inputs["qmean_w"][l].astype(np.float32)       # [S, E]
        W_fuse = (W_ps.astype(np.float64) @ qm.astype(np.float64)).astype(np.float32)  # [E, E]
        wihT = inputs["gru_wih"][l].astype(np.float32).T   # [E, 3D]
        whhT = inputs["gru_whh"][l].astype(np.float32).T   # [D, 3D]
        wqdT = inputs["qh1_w"][l][:, :D].astype(np.float32).T    # [D, E]
        wqoT = inputs["qh1_w"][l][:, D:].astype(np.float32).T    # [E(obs), E]
        wctxT = W_ctx.T                                    # [D, E]
        wfuseT = W_fuse.T                                  # [E(qh), E(h1)]

        def rz(WT):  # [K, 3D] -> rz tiles [K//128, 4, 128, 512]
            K = WT.shape[0]
            out = np.empty((K // 128, NQ, 128, 2 * QD), np.float32)
            for k in range(K // 128):
                for j in range(NQ):
                    out[k, j, :, :QD] = WT[128 * k:128 * (k + 1), QD * j:QD * (j + 1)]
                    out[k, j, :, QD:] = WT[128 * k:128 * (k + 1), D + QD * j:D + QD * (j + 1)]
            return out

        def ngate(WT):
            K = WT.shape[0]
            out = np.empty((K // 128, NQ, 128, QD), np.float32)
            for k in range(K // 128):
                for j in range(NQ):
                    out[k, j] = WT[128 * k:128 * (k + 1), 2 * D + QD * j:2 * D + QD * (j + 1)]
            return out

        d[f"wihrz{l}"] = bfc(rz(wihT))
        d[f"wihn{l}"] = bfc(ngate(wihT))
        d[f"whhrz{l}"] = bfc(rz(whhT))
        d[f"whhn{l}"] = bfc(ngate(whhT))
        d[f"wqd{l}"] = bfc(pack_quartered(wqdT))
        d[f"wfuse{l}"] = bfc(pack_quartered(wfuseT))
        d[f"wqo{l}"] = bfc(np.ascontiguousarray(wqoT.reshape(KB, 128, E)))
        if l < 2:
            d[f"wctx{l}"] = bfc(pack_quartered(wctxT))
        obs = inputs[f"obs_l{l}"].astype(np.float32)       # [B, T, E]
        d[f"obs{l}"] = bfc(obs)
    # postpass heads, packed as one [4, 8, 128, 256] (head, k, p, n): pm, ps, qm, qs
    post = np.stack([
        np.ascontiguousarray(inputs["pmean_w"][0].astype(np.float32).T.reshape(KB, 128, S)),
        np.ascontiguousarray(inputs["pstd_w"][0].astype(np.float32).T.reshape(KB, 128, S)),
        np.ascontiguousarray(inputs["qmean_w"][0].astype(np.float32).T.reshape(KB, 128, S)),
        np.ascontiguousarray(inputs["qstd_w"][0].astype(np.float32).T.reshape(KB, 128, S)),
    ])
    d["wpost"] = bfc(post)
    return d


INPUT_SPECS = None  # filled by build()


def lhs_blk(tT, k):
    """transposed-activation SBUF tile [128, 2, 128] -> lhsT block k [128, 32]"""
    s, q = k % 2, k // 2
    return tT[:, s, 32 * q:32 * (q + 1)]


def build(ctx: ExitStack, tc: tile.TileContext, outs, ins, T0=64):
    nc = tc.nc
    Ts = [T0, T0 // 4, T0 // 16]
    out_f = outs["out_f"]

    const = ctx.enter_context(tc.tile_pool(name="const", bufs=1))
    wpool = ctx.enter_context(tc.tile_pool(name="wpool", bufs=1))
    cpool = ctx.enter_context(tc.tile_pool(name="cpool", bufs=2))
    spool = ctx.enter_context(tc.tile_pool(name="spool", bufs=3))
    work = ctx.enter_context(tc.tile_pool(name="work", bufs=2))
    pps = ctx.enter_context(tc.tile_pool(name="pps", bufs=1, space="PSUM"))
    dpool = ctx.enter_context(tc.tile_pool(name="dpool", bufs=1, space="DRAM"))

    ident = const.tile([128, 128], BF16)
    make_identity(nc, ident)
    sp_bias = const.tile([128, 1], F32)
    nc.vector.memset(sp_bias, SP_BIAS)

    # DRAM intermediates (via tracked DRAM pool tiles)
    obs_part_d = [dpool.tile([Ts[l], 128, QD], F32, tag=f"obs_part{l}", name=f"obs_part{l}")
                  for l in range(3)]
    c_part_d = {l: dpool.tile([Ts[l], 128, QD], F32, tag=f"c_part{l}", name=f"c_part{l}")
                for l in range(2)}
    detT_d = [dpool.tile([Ts[l], 128, 2, 128], BF16, tag=f"detT{l}", name=f"detT{l}")
              for l in range(3)]
    qhT_d = None  # level-0 qh/det transposes stored postpass-friendly instead
    detTp_d = dpool.tile([Ts[0] // 4, 128, 2, NQ, 4, 32], BF16, tag="detTp", name="detTp")
    preact_d = dpool.tile([Ts[0] // 4, 128, 2 * S], F32, tag="preact", name="preact")
    qhTp_d = dpool.tile([Ts[0] // 4, 128, 2, NQ, 4, 32], BF16, tag="qhTp", name="qhTp")

    # ---------- obs_part emission (levels 2,1 upfront; level 0 interleaved) ----
    def emit_obs_mtile(l, m, wqo):
        obs_flat = ins[f"obs{l}"].flatten_outer_dims()      # [B*T, E]
        R = B * Ts[l]
        Tl = Ts[l]
        mrows = min(128, R - 128 * m)
        obs_n = work.tile([128, KB, 128], BF16, tag="obs_n", name="obs_n")
        nc.gpsimd.dma_start(out=obs_n[:mrows].rearrange("p k f -> p (k f)"),
                            in_=obs_flat[128 * m:128 * m + mrows, :])
        obsT_m = work.tile([128, KB, 128], BF16, tag="obsT", name="obsT_m")
        for k in range(KB):
            pt = pps.tile([128, 128], BF16, tag="p_t", bufs=2, name="pt_obs")
            nc.tensor.transpose(pt[:, :mrows], obs_n[:mrows, k, :], ident[:mrows, :mrows])
            nc.vector.tensor_copy(obsT_m[:, k, :mrows], pt[:, :mrows])
        p0 = pps.tile([128, 512], F32, tag="p_rz", name="p0", bufs=2)
        p1 = pps.tile([128, 512], F32, tag="p_h1", name="p1")
        for k in range(KB):
            nc.tensor.matmul(p0[:mrows], obsT_m[:, k, :mrows], wqo[:, k, 0:512],
                             start=(k == 0), stop=(k == KB - 1))
            nc.tensor.matmul(p1[:mrows], obsT_m[:, k, :mrows], wqo[:, k, 512:1024],
                             start=(k == 0), stop=(k == KB - 1))
        osb = work.tile([128, E], F32, tag="opart_sb", name="osb", bufs=1)
        nc.vector.tensor_copy(osb[:mrows, 0:512], p0[:mrows])
        nc.scalar.copy(osb[:mrows, 512:1024], p1[:mrows])
        nb = mrows // Tl
        b_base = (128 * m) // Tl
        dest = obs_part_d[l].rearrange("t (q b) f -> b t q f", q=NQ)
        osb_v = osb.rearrange("p (q f) -> p q f", q=NQ)
        for bb in range(nb):
            eng = nc.sync
            eng.dma_start(out=dest[b_base + bb],
                          in_=osb_v[Tl * bb:Tl * (bb + 1)])

    def load_wqo(l):
        wqo = wpool.tile([128, KB, E], BF16, tag="big1", name=f"wqo{l}")
        for k in range(KB):
            nc.gpsimd.dma_start(out=wqo[:, k, :], in_=ins[f"wqo{l}"][k])
        return wqo

    for l in (2, 1):
        wqo = load_wqo(l)
        for m in range((B * Ts[l] + 127) // 128):
            emit_obs_mtile(l, m, wqo)

    wpost_box = [None]

    post_state = {}

    def emit_post_head(m, h):
        # one head of postpass M-tile m (level-0 steps 4m..4m+3), emitted as PE
        # filler in the gates window of a scan step. pmean/qmean finish here;
        # pstd/qstd staged for the epilogue (avoids ACT table thrash mid-scan).
        wpost = wpost_box[0]
        if h == 0:
            dT4 = work.tile([128, 2, NQ, 4, 32], BF16, tag="dT4", bufs=2, name="dT4")
            qT4 = work.tile([128, 2, NQ, 4, 32], BF16, tag="qT4", bufs=2, name="qT4")
            nc.sync.dma_start(out=dT4, in_=detTp_d[m])
            nc.sync.dma_start(out=qT4, in_=qhTp_d[m])
            pa = work.tile([128, 2 * S], F32, tag="pa", bufs=2, name="pa")
            post_state[m] = (dT4, qT4, pa)
        dT4, qT4, pa = post_state[m]
        src_t = dT4 if h in (0, 1) else qT4
        ph = pps.tile([128, S], F32, tag="p_h1", name=f"post{h}")
        for k in range(KB):
            s, q = k % 2, k // 2
            lhsT = src_t[:, s, q].rearrange("p a b -> p (a b)")
            nc.tensor.matmul(ph, lhsT, wpost[:, h, k, :],
                             start=(k == 0), stop=(k == KB - 1))
        if h in (0, 2):
            hsb = work.tile([128, S], F32, tag="hsb", name="hsb")
            nc.vector.tensor_copy(hsb, ph)
            col = D if h == 0 else D + 2 * S
            dest = out_f[:, 4 * m:4 * (m + 1), col:col + S]
            nc.sync.dma_start(out=dest.rearrange("b t e -> t b e"), in_=hsb)
        else:
            nc.vector.tensor_copy(pa[:, (h // 2) * S:(h // 2) * S + S], ph)
            if h == 3:
                nc.sync.dma_start(out=preact_d[m], in_=pa)
                del post_state[m]

    # ---------- level loop ----------
    for l in (2, 1, 0):
        T = Ts[l]
        top = (l == 2)
        # c_part phase for this level (from parent's stored detT)
        if not top:
            wctx = wpool.tile([128, KB, NQ, QD], BF16, tag="big1", name=f"wctx{l}")
            for k in range(KB):
                nc.gpsimd.dma_start(out=wctx[:, k], in_=ins[f"wctx{l}"][k].rearrange("j p n -> p j n"))
            for p in range(Ts[l + 1]):
                plhsT = work.tile([128, 2, 128], BF16, tag="pstep_lhsT")
                nc.sync.dma_start(out=plhsT, in_=detT_d[l + 1][p])
                pq = pps.tile([128, QD], F32, tag="p_qh", name="pq_cp")
                for k in range(KB):
                    for j in range(NQ):
                        nc.tensor.matmul(pq[32 * j:32 * (j + 1), :], lhs_blk(plhsT, k),
                                         wctx[:, k, j, :], start=(k == 0), stop=(k == KB - 1),
                                         tile_position=(0, 32 * j), skip_group_check=True)
                csb = work.tile([128, QD], F32, tag="cpart_sb")
                nc.vector.tensor_copy(csb, pq)
                for i in range(4):
                    nc.sync.dma_start(out=c_part_d[l][4 * p + i], in_=csb)
                # keep-warm filler: cover the csb-copy + DMA-store + next
                # plhsT-load gap so HAM stays at K=8/8 through this phase
                pdumc = pps.tile([128, QD], F32, tag="p_qh", name="pdumc")
                for rep in range(2):
                    for k in range(KB):
                        for j in range(NQ):
                            nc.tensor.matmul(
                                pdumc[32 * j:32 * (j + 1), :],
                                lhs_blk(plhsT, k), wctx[:, k, j, :],
                                start=(k == 0), stop=(k == KB - 1),
                                tile_position=(0, 32 * j),
                                skip_group_check=True)
        wqo0 = None
        if l == 1:
            wqo0 = load_wqo(0)
            n_obs0_mtiles = (B * Ts[0] + 127) // 128
        if l == 0:
            wpost = wpool.tile([128, 4, KB, S], BF16, tag="big1", name="wpost")
            for h in range(4):
                for k in range(KB):
                    nc.gpsimd.dma_start(out=wpost[:, h, k, :], in_=ins["wpost"][h, k])
            wpost_box[0] = wpost

        # level weights
        wihrz = wpool.tile([128, KB, NQ, 2 * QD], BF16, tag="wihrz", name=f"wihrz{l}")
        whhrz = wpool.tile([128, KB, NQ, 2 * QD], BF16, tag="whhrz", name=f"whhrz{l}")
        wihn = wpool.tile([128, KB, NQ, QD], BF16, tag="wihn", name=f"wihn{l}")
        whhn = wpool.tile([128, KB, NQ, QD], BF16, tag="whhn", name=f"whhn{l}")
        wqd = wpool.tile([128, KB, NQ, QD], BF16, tag="wqd", name=f"wqd{l}")
        wfuse = wpool.tile([128, KB, NQ, QD], BF16, tag="wfuse", name=f"wfuse{l}")
        for k in range(KB):
            nc.gpsimd.dma_start(out=wihrz[:, k], in_=ins[f"wihrz{l}"][k].rearrange("j p n -> p j n"))
            nc.gpsimd.dma_start(out=whhrz[:, k], in_=ins[f"whhrz{l}"][k].rearrange("j p n -> p j n"))
            nc.gpsimd.dma_start(out=wihn[:, k], in_=ins[f"wihn{l}"][k].rearrange("j p n -> p j n"))
            nc.gpsimd.dma_start(out=whhn[:, k], in_=ins[f"whhn{l}"][k].rearrange("j p n -> p j n"))
            nc.gpsimd.dma_start(out=wqd[:, k], in_=ins[f"wqd{l}"][k].rearrange("j p n -> p j n"))
            nc.gpsimd.dma_start(out=wfuse[:, k], in_=ins[f"wfuse{l}"][k].rearrange("j p n -> p j n"))

        detf_c = detT_c = qhT_c = None
        qhbf_pend = None
        for t in range(T):
            first = (t == 0)
            has_ctx = not top
            if not top and l == 1 and wqo0 is not None and t < n_obs0_mtiles:
                emit_obs_mtile(0, t, wqo0)

            # stream tiles
            cpt = None
            if has_ctx:
                cpt = spool.tile([128, QD], F32, tag="cpt")
                nc.sync.dma_start(out=cpt, in_=c_part_d[l][t])
            opt = spool.tile([128, QD], F32, tag="opt")
            nc.sync.dma_start(out=opt, in_=obs_part_d[l][t])

            # --- gh first: independent of this step's chain, fills PE queue
            do_gh = not first
            prz = pgin = pghn = None
            if do_gh or not (top and first):
                prz = pps.tile([128, 2 * QD], F32, tag="p_rz", name="prz", bufs=2)
                pgin = pps.tile([128, QD], F32, tag="p_gin", name="pgin")
                pghn = pps.tile([128, QD], F32, tag="p_ghn", name="pghn")
            if do_gh:
                for k in range(KB):
                    for j in range(NQ):
                        nc.tensor.matmul(prz[32 * j:32 * (j + 1), :], lhs_blk(detT_c, k),
                                         whhrz[:, k, j, :], start=(k == 0),
                                         stop=False,
                                         tile_position=(0, 32 * j), skip_group_check=True)
                    for j in range(NQ):
                        nc.tensor.matmul(pghn[32 * j:32 * (j + 1), :], lhs_blk(detT_c, k),
                                         whhn[:, k, j, :], start=(k == 0), stop=(k == KB - 1),
                                         tile_position=(0, 32 * j), skip_group_check=True)
            # --- deferred T(qh) from previous step (behind gh in the PE queue)
            if qhbf_pend is not None:
                qhT_c = cpool.tile([128, 2, 128], BF16, tag="qhT", name="qhT_c")
                for s in range(2):
                    pt = pps.tile([128, 128], BF16, tag="p_t", bufs=2, name="pt_q")
                    nc.tensor.transpose(pt, qhbf_pend[:, 128 * s:128 * (s + 1)], ident)
                    # split the two PSUM->SBUF copies across DVE/ACT so they
                    # overlap instead of serializing on the vector engine
                    if s == 0:
                        nc.vector.tensor_copy(qhT_c[:, s, :], pt)
                    else:
                        nc.scalar.copy(qhT_c[:, s, :], pt)
                if l == 0 and t > 0:
                    nc.sync.dma_start(
                        out=qhTp_d[(t - 1) // 4][:, :, :, (t - 1) % 4, :],
                        in_=qhT_c.rearrange("p s (q b) -> p s q b", q=NQ))
                qhbf_pend = None
            # --- MM-A: h1 pre-activation from qh carry (fused weights)
            ph1 = None
            if not first:
                ph1 = pps.tile([128, QD], F32, tag="p_h1", name="ph1")
                for k in range(KB):
                    for j in range(NQ):
                        nc.tensor.matmul(ph1[32 * j:32 * (j + 1), :], lhs_blk(qhT_c, k),
                                         wfuse[:, k, j, :], start=(k == 0), stop=(k == KB - 1),
                                         tile_position=(0, 32 * j), skip_group_check=True)
            # --- h1 (bf16, relu)
            h1bf = None
            if ph1 is not None and cpt is not None:
                h1bf = work.tile([128, QD], BF16, tag="h1bf")
                nc.vector.tensor_add(h1bf, ph1, cpt)
                nc.vector.tensor_scalar_max(h1bf, h1bf, 0.0)
            elif ph1 is not None:
                h1bf = work.tile([128, QD], BF16, tag="h1bf")
                nc.vector.tensor_scalar_max(h1bf, ph1, 0.0)
            elif cpt is not None:
                h1bf = work.tile([128, QD], BF16, tag="h1bf")
                nc.vector.tensor_scalar_max(h1bf, cpt, 0.0)
            # --- T(h1)
            h1T = None
            if h1bf is not None:
                h1T = work.tile([128, 2, 128], BF16, tag="h1T")
                for s in range(2):
                    pt = pps.tile([128, 128], BF16, tag="p_t", bufs=2, name="pt_h1")
                    nc.tensor.transpose(pt, h1bf[:, 128 * s:128 * (s + 1)], ident)
                    if s == 0:
                        nc.vector.tensor_copy(h1T[:, s, :], pt)
                    else:
                        nc.scalar.copy(h1T[:, s, :], pt)

            # --- GRU (gi; gh already emitted above)
            do_gi = h1T is not None
            detf_new = cpool.tile([128, QD], F32, tag="detf")
            detbf = None
            if do_gh or do_gi:
                if do_gi:
                    for k in range(KB):
                        for j in range(NQ):
                            nc.tensor.matmul(prz[32 * j:32 * (j + 1), :], lhs_blk(h1T, k),
                                             wihrz[:, k, j, :], start=(k == 0) and not do_gh,
                                             stop=(k == KB - 1),
                                             tile_position=(0, 32 * j), skip_group_check=True)
                        for j in range(NQ):
                            nc.tensor.matmul(pgin[32 * j:32 * (j + 1), :], lhs_blk(h1T, k),
                                             wihn[:, k, j, :], start=(k == 0), stop=(k == KB - 1),
                                             tile_position=(0, 32 * j), skip_group_check=True)
                if l == 0 and t >= 4:
                    emit_post_head((t - 4) // 4, (t - 4) % 4)
                # HAM keep-warm filler: the gates' ACT/DVE chain leaves the PE
                # idle ~3us each step, re-throttling the clock gate to K=4/8
                # (all matmuls then run at 1.2GHz). Burn ~1.7us of dummy
                # streams into the recycled p_qh bank (never read; the real
                # qh matmul later start=True-clears it) to keep K=8/8.
                if detT_c is not None:
                    pdum = pps.tile([128, QD], F32, tag="p_qh", name="pdum")
                    for rep in range(2):
                        for k in range(KB):
                            for j in range(NQ):
                                nc.tensor.matmul(
                                    pdum[32 * j:32 * (j + 1), :],
                                    lhs_blk(detT_c, k),
                                    whhrz[:, k, j, 0:QD],
                                    start=(k == 0), stop=(k == KB - 1),
                                    tile_position=(0, 32 * j),
                                    skip_group_check=True)
                # gates
                r_s = work.tile([128, QD], F32, tag="r_s")
                nc.scalar.activation(r_s, prz[:, 0:QD], mybir.ActivationFunctionType.Sigmoid)
                if do_gh and do_gi:
                    t1 = work.tile([128, QD], F32, tag="t1")
                    nc.vector.tensor_mul(t1, r_s, pghn)
                    nc.vector.tensor_add(t1, t1, pgin)
                    n_in = t1
                elif do_gi:
                    n_in = pgin
                else:
                    t1 = work.tile([128, QD], F32, tag="t1")
                    nc.vector.tensor_mul(t1, r_s, pghn)
                    n_in = t1
                z_s = work.tile([128, QD], F32, tag="z_s")
                nc.scalar.activation(z_s, prz[:, QD:2 * QD], mybir.ActivationFunctionType.Sigmoid)
                n_s = work.tile([128, QD], F32, tag="n_s")
                nc.scalar.activation(n_s, n_in, mybir.ActivationFunctionType.Tanh)
                # det' = n*(1-z) + z*det ; omz/zdet overlap the tanh
                omz = work.tile([128, QD], F32, tag="omz")
                nc.vector.tensor_scalar(omz, z_s, -1.0, 1.0,
                                        mybir.AluOpType.mult, mybir.AluOpType.add)
                d1 = work.tile([128, QD], F32, tag="d1")
                if not first:
                    nc.vector.tensor_mul(d1, z_s, detf_c)
                    nc.vector.tensor_mul(omz, omz, n_s)
                    nc.vector.tensor_add(detf_new, omz, d1)
                else:
                    nc.vector.tensor_mul(detf_new, omz, n_s)
                detbf = work.tile([128, QD], BF16, tag="detbf")
                nc.vector.tensor_copy(detbf, detf_new)
            else:
                nc.vector.memset(detf_new, 0.0)

            # --- T(det)
            detT_new = cpool.tile([128, 2, 128], BF16, tag="detT")
            if detbf is not None:
                for s in range(2):
                    pt = pps.tile([128, 128], BF16, tag="p_t", bufs=2, name="pt_d")
                    nc.tensor.transpose(pt, detbf[:, 128 * s:128 * (s + 1)], ident)
                    if s == 0:
                        nc.vector.tensor_copy(detT_new[:, s, :], pt)
                    else:
                        nc.scalar.copy(detT_new[:, s, :], pt)
            else:
                nc.vector.memset(detT_new, 0.0)
            if l == 0:
                nc.sync.dma_start(
                    out=detTp_d[t // 4][:, :, :, t % 4, :],
                    in_=detT_new.rearrange("p s (q b) -> p s q b", q=NQ))
                dest = out_f[:, t, 0:D].rearrange("b (q f) -> q b f", q=NQ)
                nc.sync.dma_start(out=dest, in_=detf_new)
            else:
                nc.sync.dma_start(out=detT_d[l][t], in_=detT_new)

            # --- qh
            pqh = None
            if detbf is not None:
                pqh = pps.tile([128, QD], F32, tag="p_qh", name="pqh")
                for k in range(KB):
                    for j in range(NQ):
                        nc.tensor.matmul(pqh[32 * j:32 * (j + 1), :], lhs_blk(detT_new, k),
                                         wqd[:, k, j, :], start=(k == 0), stop=(k == KB - 1),
                                         tile_position=(0, 32 * j), skip_group_check=True)
            qhbf = work.tile([128, QD], BF16, tag="qhbf")
            if pqh is not None:
                nc.vector.tensor_add(qhbf, pqh, opt)
                nc.vector.tensor_scalar_max(qhbf, qhbf, 0.0)
            else:
                nc.vector.tensor_scalar_max(qhbf, opt, 0.0)
            qhbf_pend = qhbf
            detf_c, detT_c = detf_new, detT_new

    # final deferred T(qh) of the last level-0 step (feeds qhTp store)
    if qhbf_pend is not None:
        qhT_c = cpool.tile([128, 2, 128], BF16, tag="qhT", name="qhT_last")
        for s in range(2):
            pt = pps.tile([128, 128], BF16, tag="p_t", bufs=2, name="pt_ql")
            nc.tensor.transpose(pt, qhbf_pend[:, 128 * s:128 * (s + 1)], ident)
            nc.vector.tensor_copy(qhT_c[:, s, :], pt)
        nc.sync.dma_start(
            out=qhTp_d[(Ts[0] - 1) // 4][:, :, :, (Ts[0] - 1) % 4, :],
            in_=qhT_c.rearrange("p s (q b) -> p s q b", q=NQ))
    for h in range(4):
        emit_post_head(Ts[0] // 4 - 1, h)

    # ---------- level-0 heads epilogue: softplus of staged pre-activations ----
    for m in range(Ts[0] // 4):
        pa = work.tile([128, 2 * S], F32, tag="pa_e")
        nc.sync.dma_start(out=pa, in_=preact_d[m])
        pe_ = work.tile([128, 2 * S], F32, tag="pe_e")
        nc.scalar.activation(pe_, pa, mybir.ActivationFunctionType.Exp, bias=sp_bias)
        nc.scalar.activation(pe_, pe_, mybir.ActivationFunctionType.Ln, bias=1.0)
        nc.vector.tensor_scalar_add(pe_, pe_, MIN_STD)
        d0 = out_f[:, 4 * m:4 * (m + 1), D + S:D + 2 * S]
        nc.sync.dma_start(out=d0.rearrange("b t e -> t b e"), in_=pe_[:, 0:S])
        d1_ = out_f[:, 4 * m:4 * (m + 1), D + 3 * S:D + 4 * S]
        nc.sync.dma_start(out=d1_.rearrange("b t e -> t b e"), in_=pe_[:, S:2 * S])


# ------------------------- runner -------------------------
_CACHE = {}


def _get_program(T0):
    if T0 in _CACHE:
        return _CACHE[T0]
    from concourse import bacc
    nc = bacc.Bacc("TRN2", target_bir_lowering=False, debug=False, num_devices=1)
    in_specs = _input_specs(T0)
    ins = {k: nc.dram_tensor(k, list(shape), dt, kind="ExternalInput").ap()
           for k, (shape, dt) in in_specs.items()}
    outs = {"out_f": nc.dram_tensor("out_f", [B, T0, D + 4 * S], F32,
                                    kind="ExternalOutput").ap()}
    with tile.TileContext(nc) as tc:
        with ExitStack() as ctx:
            build(ctx, tc, outs, ins, T0=T0)
    nc.compile()
    _CACHE[T0] = nc
    return nc


def _input_specs(T0):
    Ts = [T0, T0 // 4, T0 // 16]
    sp = {}
    for l in range(3):
        sp[f"wihrz{l}"] = ([KB, NQ, 128, 2 * QD], BF16)
        sp[f"wihn{l}"] = ([KB, NQ, 128, QD], BF16)
        sp[f"whhrz{l}"] = ([KB, NQ, 128, 2 * QD], BF16)
        sp[f"whhn{l}"] = ([KB, NQ, 128, QD], BF16)
        sp[f"wqd{l}"] = ([KB, NQ, 128, QD], BF16)
        sp[f"wfuse{l}"] = ([KB, NQ, 128, QD], BF16)
        sp[f"wqo{l}"] = ([KB, 128, E], BF16)
        if l < 2:
            sp[f"wctx{l}"] = ([KB, NQ, 128, QD], BF16)
        sp[f"obs{l}"] = ([B, Ts[l], E], BF16)
    sp["wpost"] = ([4, KB, 128, S], BF16)
    return sp


def run(inputs, trace=False):
    from concourse.bass_utils import run_bass_kernel_spmd
    inputs = {k: np.asarray(v) for k, v in inputs.items()}
    T0 = int(inputs["obs_l0"].shape[1])
    prepped = prep_inputs(inputs, T0)
    nc = _get_program(T0)
    res = run_bass_kernel_spmd(nc, [prepped], core_ids=[0], trace=trace)
    out = res.results[0]["out_f"].astype(np.float32)
    return out, res


def kernel(**inputs):
    out, _ = run(inputs, trace=False)
    return out

